# revision 7
# baseline (speedup 1.0000x reference)
"""Trainium2 Bass kernel for AttentionAlignmentLoss.

Math (matches the jax reference):
  s = clip(floor(ts0*12.5), 0, F-1); e = max(s+1, min(floor(ts1*12.5)+1, F))
  gt[f] = min((f-s+5)/5, (e+4-f)/5, 1) clamped at 0   (trapezoid; verified
          identical to the reference's core/up/down construction)
  loss  = sum((1 - <pred,gt>/(max(|pred|,eps)*|gt|)) * mask) / max(sum(mask),1)

Device mapping (per core, batch-sharded 2 of 16):
  1024 rows x F=3000.  8 groups of 128 partitions.
  |gt|^2 is computed analytically from (s,e) -- no big-tensor pass needed:
      |gt|^2 = (e-s) + g(min(4,s)) + g(min(4,F-e)),  g(n)=n(2n^2-27n+121)/150
  Per group, big passes over [128,3000]:
      ACT: AB = Abs(2f - (s+e-1))          (out bf16)
      DVE: m1 = min(AB - k, 0), k=e-s+9    (bf16, 4x mode)
      DVE: STT out=(m1 max -10)*pred, accum=dot_raw   (dot = -0.1*dot_raw)
      ACT: Square(pred), accum=sum(pred^2)
  Host: sum 8x[128,2] partials, loss = L/max(C,1).
"""

import numpy as np
from contextlib import ExitStack

N_CORES = 8
B, T, F = 16, 512, 3000
B_SH = B // N_CORES          # 2 batches per core
ROWS = B_SH * T              # 1024 rows per core
G = ROWS // 128              # 8 groups of 128 partitions

_CACHE = {}


def _build_module(variant='full'):
    import concourse.bacc as bacc
    import concourse.tile as tile
    from concourse import mybir

    fp32 = mybir.dt.float32
    bf16 = mybir.dt.bfloat16
    i32 = mybir.dt.int32
    AF = mybir.ActivationFunctionType
    OP = mybir.AluOpType
    AX = mybir.AxisListType

    nc = bacc.Bacc("TRN2", target_bir_lowering=False, debug=False)

    pred_d = nc.dram_tensor("pred", [ROWS, F], fp32, kind="ExternalInput").ap()
    ts_d = nc.dram_tensor("ts", [G, 128, 2], fp32, kind="ExternalInput").ap()
    mask_d = nc.dram_tensor("mask", [G, 128, 1], fp32, kind="ExternalInput").ap()
    out_d = nc.dram_tensor("out", [128, 2], fp32, kind="ExternalOutput").ap()

    with tile.TileContext(nc) as tc, ExitStack() as ctx:
        const_pool = ctx.enter_context(tc.tile_pool(name="const", bufs=1))
        pred_pool = ctx.enter_context(tc.tile_pool(name="predp", bufs=3))
        ab_pool = ctx.enter_context(tc.tile_pool(name="abp", bufs=2))
        m1_pool = ctx.enter_context(tc.tile_pool(name="m1p", bufs=2))
        scr_pool = ctx.enter_context(tc.tile_pool(name="scrp", bufs=1))
        small = ctx.enter_context(tc.tile_pool(name="small", bufs=1))

        _sn = [0]

        def stile(shape, dt=fp32):
            _sn[0] += 1
            return small.tile(shape, dt, name=f"sm{_sn[0]}")

        # ---- constant: f2[p, f] = 2*f (fp32, same every partition) ----
        f2_i = const_pool.tile([128, F], i32)
        if variant == "noiota":
            nc.vector.memset(f2_i[:], 0)
        else:
            nc.gpsimd.iota(f2_i[:], pattern=[[2, F]], base=0, channel_multiplier=0)
        f2 = const_pool.tile([128, F], fp32)
        nc.vector.tensor_copy(f2[:], f2_i[:])

        # ---- per-row scalars for all 8 groups at once ----
        ts_t = stile([128, G, 2])
        nc.sync.dma_start(ts_t[:], ts_d.rearrange("g p c -> p g c"))
        mask_t = stile([128, G])
        nc.sync.dma_start(mask_t[:], mask_d.rearrange("g p one -> p (g one)"))

        mm = stile([128, G, 2])
        nc.vector.tensor_scalar(mm[:], ts_t[:], 12.5, None, OP.mult)
        # floor(mm): int cast (any rounding within 1) then fix up with is_gt
        fc_i = stile([128, G, 2], i32)
        nc.vector.tensor_copy(fc_i[:], mm[:])
        fcf = stile([128, G, 2])
        nc.vector.tensor_copy(fcf[:], fc_i[:])
        gt1 = stile([128, G, 2])
        nc.vector.tensor_tensor(gt1[:], fcf[:], mm[:], OP.is_gt)
        fl = stile([128, G, 2])
        nc.vector.tensor_tensor(fl[:], fcf[:], gt1[:], OP.subtract)  # floor

        s_t = stile([128, G])
        nc.vector.tensor_scalar(s_t[:], fl[:, :, 0], 0.0, float(F - 1), OP.max, OP.min)
        e1 = stile([128, G])
        nc.vector.tensor_scalar(e1[:], fl[:, :, 1], 1.0, float(F), OP.add, OP.min)
        sp1 = stile([128, G])
        nc.vector.tensor_scalar(sp1[:], s_t[:], 1.0, None, OP.add)
        e_t = stile([128, G])
        nc.vector.tensor_tensor(e_t[:], e1[:], sp1[:], OP.max)

        # negc = 1 - (s+e);  bias for ACT Abs: |2f + negc... wait c=s+e-1,
        # we need |2f - c| = Abs(2f + (1-s-e))
        c1 = stile([128, G])
        nc.vector.tensor_tensor(c1[:], s_t[:], e_t[:], OP.add)
        negc = stile([128, G])
        nc.vector.tensor_scalar(negc[:], c1[:], 1.0, -1.0, OP.subtract, OP.mult)

        d0 = stile([128, G])  # e - s
        nc.vector.tensor_tensor(d0[:], e_t[:], s_t[:], OP.subtract)
        k_t = stile([128, G])  # k = e - s + 9
        nc.vector.tensor_scalar(k_t[:], d0[:], 9.0, None, OP.add)

        # ---- analytic |gt|^2 = (e-s) + g(n1) + g(n2) ----
        n1 = stile([128, G])
        nc.vector.tensor_scalar(n1[:], s_t[:], 4.0, None, OP.min)
        t30 = stile([128, G])
        nc.vector.tensor_scalar(t30[:], e_t[:], float(F), -1.0, OP.subtract, OP.mult)
        n2 = stile([128, G])
        nc.vector.tensor_scalar(n2[:], t30[:], 4.0, None, OP.min)

        def gpoly(n_ap):
            # g(n) = n * (n^2 - 13.5 n + 60.5) / 75
            nn = stile([128, G])
            nc.vector.tensor_tensor(nn[:], n_ap, n_ap, OP.mult)
            v = stile([128, G])
            nc.vector.tensor_scalar(v[:], n_ap, 13.5, None, OP.mult)
            w = stile([128, G])
            nc.vector.tensor_tensor(w[:], nn[:], v[:], OP.subtract)
            y = stile([128, G])
            nc.vector.tensor_scalar(y[:], w[:], 60.5, 1.0 / 75.0, OP.add, OP.mult)
            up = stile([128, G])
            nc.vector.tensor_tensor(up[:], y[:], n_ap, OP.mult)
            return up

        up1 = gpoly(n1[:])
        up2 = gpoly(n2[:])
        g1 = stile([128, G])
        nc.vector.tensor_tensor(g1[:], d0[:], up1[:], OP.add)
        gn2 = stile([128, G])
        nc.vector.tensor_tensor(gn2[:], g1[:], up2[:], OP.add)
        gn = stile([128, G])
        nc.scalar.activation(gn[:], gn2[:], AF.Sqrt)

        # ---- main loop over 8 groups ----
        dots = stile([128, G])
        psq = stile([128, G])
        for g in range(G):
            pt = pred_pool.tile([128, F], fp32, tag="pt")
            nc.sync.dma_start(pt[:], pred_d[g * 128:(g + 1) * 128, :])

            ab = ab_pool.tile([128, F], bf16, tag="ab")
            nc.scalar.activation(ab[:], f2[:], AF.Abs, bias=negc[:, g:g + 1], scale=1.0)

            m1 = m1_pool.tile([128, F], bf16, tag="m1")
            nc.vector.tensor_scalar(
                m1[:], ab[:], k_t[:, g:g + 1], 0.0, OP.subtract, OP.min
            )

            scr = scr_pool.tile([128, F], fp32, tag="scr")
            nc.vector.scalar_tensor_tensor(
                scr[:], m1[:], -10.0, pt[:], OP.max, OP.mult,
                accum_out=dots[:, g:g + 1],
            )

            scr2 = scr_pool.tile([128, F], fp32, tag="scr2")
            nc.scalar.activation(scr2[:], pt[:], AF.Square, accum_out=psq[:, g:g + 1])

        # ---- finalize: per-row loss, accumulate per partition ----
        pn_r = stile([128, G])
        nc.scalar.activation(pn_r[:], psq[:], AF.Sqrt)
        pn = stile([128, G])
        nc.vector.tensor_scalar(pn[:], pn_r[:], 1e-8, None, OP.max)
        den = stile([128, G])
        nc.vector.tensor_tensor(den[:], pn[:], gn[:], OP.mult)
        rec = stile([128, G])
        if variant == "norecip":
            nc.vector.tensor_copy(rec[:], den[:])
        else:
            nc.vector.reciprocal(rec[:], den[:])
        cosr = stile([128, G])  # cos / (-0.1)
        nc.vector.tensor_tensor(cosr[:], dots[:], rec[:], OP.mult)
        om = stile([128, G])  # 1 - cos = 1 + 0.1*cosr
        nc.vector.tensor_scalar(om[:], cosr[:], 0.1, 1.0, OP.mult, OP.add)
        lt = stile([128, G])
        nc.vector.tensor_tensor(lt[:], om[:], mask_t[:], OP.mult)

        outt = stile([128, 2])
        nc.vector.tensor_reduce(outt[:, 0:1], lt[:], AX.X, OP.add)
        nc.vector.tensor_reduce(outt[:, 1:2], mask_t[:], AX.X, OP.add)
        nc.sync.dma_start(out_d[:], outt[:])

    nc.compile()
    return nc


def _get_module():
    if "nc" not in _CACHE:
        _CACHE["nc"] = _build_module()
    return _CACHE["nc"]


def _in_maps(predicted_attn, token_timestamps, attention_mask):
    maps = []
    for i in range(N_CORES):
        b0, b1 = i * B_SH, (i + 1) * B_SH
        pred_i = np.ascontiguousarray(
            predicted_attn[b0:b1].reshape(ROWS, F), dtype=np.float32
        )
        ts_i = np.ascontiguousarray(
            token_timestamps[b0:b1].reshape(G, 128, 2), dtype=np.float32
        )
        mask_i = np.ascontiguousarray(
            attention_mask[b0:b1].reshape(G, 128, 1), dtype=np.float32
        )
        maps.append({"pred": pred_i, "ts": ts_i, "mask": mask_i})
    return maps


def _finish(results):
    L = 0.0
    C = 0.0
    for r in results:
        L += float(r["out"][:, 0].sum(dtype=np.float64))
        C += float(r["out"][:, 1].sum(dtype=np.float64))
    return np.float32(L / max(C, 1.0))


def kernel(predicted_attn, token_timestamps, attention_mask):
    from concourse.bass_utils import run_bass_kernel_spmd

    nc = _get_module()
    maps = _in_maps(
        np.asarray(predicted_attn), np.asarray(token_timestamps),
        np.asarray(attention_mask),
    )
    res = run_bass_kernel_spmd(nc, maps, core_ids=list(range(N_CORES)))
    return _finish(res.results)


def _install_ntff_shim():
    """Provide antenv.axon_hooks (absent in this image) so trace=True works,
    driving NTFF capture via ctypes into libaxon_pjrt.so. Test-time only."""
    import sys
    import types
    import ctypes
    import contextlib

    if "antenv.axon_hooks" in sys.modules:
        return
    so_path = "/opt/axon/libaxon_pjrt.so"
    lib = ctypes.CDLL(so_path)
    if not hasattr(lib, "axon_start_nrt_profile"):
        return
    lib.axon_start_nrt_profile.argtypes = [
        ctypes.POINTER(ctypes.c_int64), ctypes.c_size_t,
    ]
    lib.axon_start_nrt_profile.restype = ctypes.c_int64
    lib.axon_stop_nrt_profile.argtypes = [ctypes.c_char_p]
    lib.axon_stop_nrt_profile.restype = ctypes.c_int64

    @contextlib.contextmanager
    def _hook(output_dir, device_ids):
        import jax

        jax.devices()
        if device_ids:
            ids = (ctypes.c_int64 * len(device_ids))(*device_ids)
            rc = lib.axon_start_nrt_profile(ids, len(device_ids))
        else:
            rc = lib.axon_start_nrt_profile(None, 0)
        if rc != 0:
            raise RuntimeError(f"axon_start_nrt_profile rc={rc}")
        try:
            yield
        finally:
            n = lib.axon_stop_nrt_profile(str(output_dir).encode())
            print(f"ntff profile: {n} file(s) written to {output_dir}")

    mod = types.ModuleType("antenv.axon_hooks")
    _h = [_hook]
    mod.get_axon_ntff_profile_hook = lambda: _h[0]
    mod.set_axon_ntff_profile_hook = lambda h: _h.__setitem__(0, h)
    sys.modules["antenv.axon_hooks"] = mod
    import antenv

    antenv.axon_hooks = mod


def kernel_profiled(predicted_attn, token_timestamps, attention_mask, tmpdir=None):
    """Same as kernel() but requests an NTFF trace; returns (loss, exec_ns, res)."""
    from concourse import bass_utils
    from concourse.bass_utils import run_bass_kernel_spmd

    _install_ntff_shim()
    bass_utils.upload_artifacts = lambda tmpdir: str(tmpdir)  # no S3 here

    nc = _get_module()
    maps = _in_maps(
        np.asarray(predicted_attn), np.asarray(token_timestamps),
        np.asarray(attention_mask),
    )
    res = run_bass_kernel_spmd(
        nc, maps, core_ids=list(range(N_CORES)), trace=True, tmpdir=tmpdir
    )
    return _finish(res.results), res.exec_time_ns, res
